# revision 30
# baseline (speedup 1.0000x reference)
"""Multi-head causal+padded attention on 8 Trainium2 NeuronCores.

Core c handles batch b = c//2 and head-group g = c%2 (8 of 16 heads).

Pad compaction: the reference masks out padded keys/queries entirely
(padded query rows output 0). Attention over the pad-compacted sequence is
exactly equivalent, so the host gathers the ~1024 unpadded rows per batch,
zero-pads to a fixed 1088 capacity, and the device runs a causal MHA on
[1088]. Outputs are scattered back with zeros in padded rows.

Device (per core, all-bf16 datapath, fp32 PSUM):
  qT/kT = W^T-slices @ xT in [out, seq] layout; v in natural [seq, out]
  layout augmented with a ones column (softmax denominator rides along the
  att@v accumulation chain). Scores transposed per 128-k-block, head pairs
  packed into PE row-groups 0-1/2-3 (concurrent matmuls); consecutive
  k-blocks share PSUM tiles so one batched exp serves several blocks;
  tri-masked on GpSimd, then att^T-chained into [65, CH] PSUM; DVE
  evacuates unnormalized out+denominator as bf16; the host divides.

  Startup: input DMA fans out over the three DMA-capable queues (x split
  over sync+gpsimd, wk/wq interleaved by eb on scalar so the paired k+q
  ob-chains stream at DMA pace); the per-partition q/k biases arrive as
  one contiguous [8,128] row-DMA and are PE-transposed on the otherwise
  idle tensor engine. Upfront projection groups pair k+q chains per
  ob-block so the first score pair (which only needs the ob0 slices)
  starts as early as possible. Emission is software-pipelined with stage
  offsets: slot j emits [proj chains | exp+mask(j+1) | scores(j+2)
  interleaved with av(j) | out(j)].
"""
import sys

sys.path.insert(0, "/opt/trn_rl_repo")

import numpy as np

E = 1024
D = 64
HPC = 8         # heads per core
OC = HPC * D    # 512 output dims per core
EB = E // 128   # 8 contraction blocks
B = 4
NCORES = 8
CH = 512        # max q-chunk width (PSUM-bank exact)
CAP0 = 1088     # default compacted seq capacity (multiple of 64)

_cache = {}


def _qchunks(seqc):
    out, q0 = [], 0
    while q0 < seqc:
        w = min(CH, seqc - q0)
        out.append((q0, w))
        q0 += w
    return out


def _kblocks(seqc):
    out, k0 = [], 0
    while k0 < seqc:
        w = min(128, seqc - k0)
        out.append((k0, w))
        k0 += w
    return out


def _build_nc(seqc):
    from concourse import bacc
    import concourse.tile as tile
    import concourse.mybir as mybir

    assert seqc % 64 == 0
    QC = _qchunks(seqc)          # [(q0, cw)]
    KB = _kblocks(seqc)          # [(k0, kw)]
    NCH = len(QC)
    NB = len(KB)
    F32 = mybir.dt.float32
    BF16 = mybir.dt.bfloat16
    AF = mybir.ActivationFunctionType

    nc = bacc.Bacc("TRN2", target_bir_lowering=False, debug=False,
                   num_devices=NCORES)
    xT = nc.dram_tensor("xT", [E, seqc], BF16, kind="ExternalInput").ap()
    wqT = nc.dram_tensor("wqT", [E, OC], BF16, kind="ExternalInput").ap()
    wkT = nc.dram_tensor("wkT", [E, OC], BF16, kind="ExternalInput").ap()
    wvT = nc.dram_tensor("wvT", [E, OC], BF16, kind="ExternalInput").ap()
    # bq/bk stacked as [8, 128]: rows 0-3 = bq.reshape(4,128), 4-7 = bk
    bqk = nc.dram_tensor("bqk", [8, 128], F32, kind="ExternalInput").ap()
    bv = nc.dram_tensor("bv", [OC], F32, kind="ExternalInput").ap()
    # unnormalized out (64 rows) + denominator (row 64) per head
    outT = nc.dram_tensor("outT", [HPC * 65, seqc], BF16,
                          kind="ExternalOutput").ap()

    with tile.TileContext(nc) as tc:
        with tc.tile_pool(name="const", bufs=1) as cpool, \
             tc.tile_pool(name="big", bufs=1) as bigpool, \
             tc.tile_pool(name="psP", bufs=2, space="PSUM") as psP, \
             tc.tile_pool(name="psS", bufs=2, space="PSUM") as psS, \
             tc.tile_pool(name="psAv", bufs=2, space="PSUM") as psAv, \
             tc.tile_pool(name="att", bufs=5) as att_pool, \
             tc.tile_pool(name="outp", bufs=6) as out_pool:

            # ---------------- persistent SBUF ----------------
            x_sb = bigpool.tile([128, EB * seqc], BF16, tag="x_sb")
            wq_sb = bigpool.tile([128, EB * OC], BF16, tag="wq_sb")
            wk_sb = bigpool.tile([128, EB * OC], BF16, tag="wk_sb")
            wv_sb = bigpool.tile([128, EB * OC], BF16, tag="wv_sb")
            qT_sb = bigpool.tile([128, 4 * seqc], BF16, tag="qT")
            kT_sb = bigpool.tile([128, 4 * seqc], BF16, tag="kT")
            v_aug = bigpool.tile([128, NB * HPC * 65], BF16, tag="v_aug")
            v_r = v_aug[:].rearrange("p (b h c) -> p b h c", b=NB, h=HPC)

            # tri[k, q] = 1 where k <= q else 0 (diagonal 128x128 block);
            # memset early so the warm exp below has a ready operand
            tri = cpool.tile([128, 128], BF16, tag="tri")
            nc.gpsimd.memset(tri[:], 1.0)
            # ~4us of dummy matmuls while the input DMA streams: HAM
            # un-throttles the PE clock (1.2 -> 2.4 GHz) after ~3.4us of
            # sustained activity, so the real chains start warm
            dumm = cpool.tile([128, 512], BF16, tag="dumm")
            nc.gpsimd.memset(dumm[:], 0.0)
            warm_ps = psP.tile([128, 512], F32, tag="ps_proj", name="warm")
            for _ in range(10):
                nc.tensor.matmul(warm_ps[:], dumm[:, 0:128], dumm[:],
                                 start=True, stop=True)
            # hoist the exp ACT_TABLE_LOAD (~2.7us) into the DMA window;
            # scale=0.0 makes the operand value irrelevant
            warm = cpool.tile([1, 4], F32, tag="warm")
            nc.scalar.activation(warm[:], tri[0:1, 0:4], AF.Exp, scale=0.0)
            # 8x8 identity for the PE-transpose of the stacked q/k biases
            id8 = cpool.tile([8, 8], F32, tag="id8")
            nc.gpsimd.memset(id8[:], 0.0)
            nc.gpsimd.affine_select(
                out=id8[:], in_=id8[:],
                compare_op=mybir.AluOpType.not_equal, fill=1.0,
                base=0, pattern=[[-1, 8]], channel_multiplier=1)

            # The upfront 8-chain group consumes (x, wk, wq)[eb] per eb
            # step, so the gating 4.36MB is spread evenly over the three
            # DMA queues roughly in eb order; wv (not needed until the
            # first v chains, several slots in) strictly follows.
            def dma_w(eng, w_sb, wT, eb):
                eng.dma_start(w_sb[:, eb * OC:(eb + 1) * OC],
                              wT[eb * 128:(eb + 1) * 128, :])

            def dma_x(eng, eb):
                eng.dma_start(x_sb[:, eb * seqc:(eb + 1) * seqc],
                              xT[eb * 128:(eb + 1) * 128, :])

            def dma_xh(eng, eb, half):
                c0, c1 = (0, 512) if half == 0 else (512, seqc)
                eng.dma_start(x_sb[:, eb * seqc + c0:eb * seqc + c1],
                              xT[eb * 128:(eb + 1) * 128, c0:c1])

            # gating set (x cols 0:512 + wk + wq = what the upfront ch0
            # chains consume) round-robins the three queues in eb order
            # so per-eb triples land together; the rest streams behind
            ENG = (nc.sync, nc.scalar, nc.gpsimd)
            bqk_sb = cpool.tile([8, 128], F32, tag="bqk")
            nc.sync.dma_start(bqk_sb[:], bqk)
            for eb in range(EB):
                dma_xh(ENG[eb % 3], eb, 0)
                dma_w(ENG[(eb + 1) % 3], wk_sb, wkT, eb)
                dma_w(ENG[(eb + 2) % 3], wq_sb, wqT, eb)
            bv_row = cpool.tile([1, OC], F32, tag="bv_row")
            nc.sync.dma_start(bv_row[:], bv.rearrange("(a c) -> a c", a=1))
            for eb in range(EB):
                dma_xh(ENG[(eb + 1) % 3], eb, 1)
                dma_w(ENG[(eb + 2) % 3], wv_sb, wvT, eb)

            # biases: PE-transpose [8,128] -> [128,8] on the idle PE queue
            bqk_T = cpool.tile([128, 8], F32, tag="bqkT")
            ps_b = psP.tile([128, 512], F32, tag="ps_proj", name="ps_b")
            nc.tensor.transpose(ps_b[:, 0:8], bqk_sb[:], id8[:])
            nc.vector.tensor_copy(bqk_T[:], ps_b[:, 0:8])
            bq_sb = bqk_T[:, 0:4]
            bk_sb = bqk_T[:, 4:8]

            nc.gpsimd.affine_select(
                out=tri[:], in_=tri[:], compare_op=mybir.AluOpType.is_ge,
                fill=0.0, base=0, pattern=[[1, 128]], channel_multiplier=-1)
            bv_tile = cpool.tile([128, OC], F32, tag="bv_tile")
            nc.gpsimd.partition_broadcast(bv_tile[:], bv_row[:])
            nc.gpsimd.memset(v_r[:, :, :, 64], 1.0)

            # ---------------- projection chain groups ----------------
            # chain spec: ("k"|"q", ob, ch) weight-stationary, or ("v", sb)
            def emit_group(chains):
                # PSUM banks per chain: 2x psP, then loans of psS pair
                # tiles (2 banks each) and psAv tiles for the startup
                # 8-chain group; attention hasn't started yet then
                pss = []
                loans = {}
                for idx, c in enumerate(chains):
                    if idx < 2:
                        ps = psP.tile([128, 512], F32, tag="ps_proj",
                                      name=f"pp{idx}")
                    elif idx < 6:
                        li = (idx - 2) // 2
                        if li not in loans:
                            loans[li] = psS.tile([128, 1024], F32,
                                                 tag="ps_s", name="loan")
                        ps = loans[li][:, (idx % 2) * 512:
                                       (idx % 2 + 1) * 512]
                    else:
                        ps = psAv.tile([128, 512], F32, tag="ps_av",
                                       name=f"av{idx - 6}")
                    pss.append(ps)
                for eb in range(EB):
                    for c, ps in zip(chains, pss):
                        if c[0] == "v":
                            sb = c[1]
                            k0, kw = KB[sb]
                            nc.tensor.matmul(
                                ps[0:kw, :],
                                x_sb[:, eb * seqc + k0:eb * seqc + k0 + kw],
                                wv_sb[:, eb * OC:(eb + 1) * OC],
                                start=(eb == 0), stop=(eb == EB - 1))
                        else:
                            _, ob, ch = c
                            q0, cw = QC[ch]
                            w_sb = wk_sb if c[0] == "k" else wq_sb
                            nc.tensor.matmul(
                                ps[:, 0:cw],
                                w_sb[:, eb * OC + ob * 128:
                                     eb * OC + (ob + 1) * 128],
                                x_sb[:, eb * seqc + q0:eb * seqc + q0 + cw],
                                start=(eb == 0), stop=(eb == EB - 1))
                for c, ps in zip(chains, pss):
                    if c[0] == "v":
                        sb = c[1]
                        k0, kw = KB[sb]
                        nc.vector.tensor_add(
                            v_r[0:kw, sb, :, 0:64],
                            ps[0:kw, :].rearrange("p (h c) -> p h c", h=HPC),
                            bv_tile[0:kw, :].rearrange("p (h c) -> p h c",
                                                       h=HPC))
                    else:
                        _, ob, ch = c
                        q0, cw = QC[ch]
                        dst = kT_sb if c[0] == "k" else qT_sb
                        bias_sb = bk_sb if c[0] == "k" else bq_sb
                        nc.vector.tensor_scalar_add(
                            dst[:, ob * seqc + q0:ob * seqc + q0 + cw],
                            ps[:, 0:cw], bias_sb[:, ob:ob + 1])

            # ---------------- attention emitters ----------------
            def widths(scn):
                """[(kb, off, iw, lstart)] for chunk scn, packed offsets."""
                q0, cw = QC[scn]
                out, off = [], 0
                for kb, (k0, kw) in enumerate(KB):
                    if k0 >= q0 + cw:
                        break
                    lstart = max(0, k0 - q0)
                    iw = cw - lstart
                    out.append((kb, off, iw, lstart))
                    off += iw
                return out

            def groups_of(items):
                """Pack consecutive items into <=512-col groups sharing a
                PSUM tile + one exp; break when the k-width changes."""
                gs, cur, w = [], [], 0
                for it in items:
                    kw = KB[it[0]][1]
                    if cur and (w + it[2] > 512 or KB[cur[0][0]][1] != kw):
                        gs.append(cur)
                        cur, w = [], 0
                    cur.append(it)
                    w += it[2]
                if cur:
                    gs.append(cur)
                return gs

            state = {}

            def emit_score_group(p, att, sw, grp):
                """Paired score MMs per item (head pair -> the two banks of
                one PSUM tile), then ONE fused strided exp PSUM->SBUF for
                the whole group, then gpsimd tri-masks on diagonal items."""
                scn, hp = p
                q0, cw = QC[scn]
                kw = KB[grp[0][0]][1]
                gw = sum(it[2] for it in grp)
                off0 = grp[0][1]
                ssb = psS.tile([128, 1024], F32, tag="ps_s")
                goff = 0
                for kb, off, iw, lstart in grp:
                    k0, _ = KB[kb]
                    for i in range(2):
                        h = 2 * hp + i
                        ob, po = h // 2, (h % 2) * 64
                        # one accumulation epoch per bank: start only on the
                        # group's first item (first_mm clears the whole
                        # bank's has_written bits), stop on its last
                        nc.tensor.matmul(
                            ssb[0:kw, i * 512 + goff:i * 512 + goff + iw],
                            kT_sb[po:po + 64,
                                  ob * seqc + k0:ob * seqc + k0 + kw],
                            qT_sb[po:po + 64,
                                  ob * seqc + q0 + lstart:
                                  ob * seqc + q0 + cw],
                            start=(goff == 0), stop=(goff + iw == gw))
                    goff += iw
                src = ssb[:].rearrange("p (i c) -> p i c", i=2)[0:kw, :, 0:gw]
                dst = att[:].rearrange("p (i c) -> p i c",
                                       i=2)[0:kw, :, off0:off0 + gw]
                nc.scalar.activation(dst, src, AF.Exp, scale=0.125)
                for kb, off, iw, lstart in grp:
                    k0, bkw = KB[kb]
                    if k0 >= q0:  # diagonal block: causal tri mask
                        mw = min(bkw, iw)
                        for i in range(2):
                            nc.gpsimd.tensor_mul(
                                att[0:bkw, i * sw + off:i * sw + off + mw],
                                att[0:bkw, i * sw + off:i * sw + off + mw],
                                tri[0:bkw, 0:mw])

            def emit_av_kb(p, att, sw, avs, item, nkb):
                scn, hp = p
                q0, cw = QC[scn]
                kb, off, iw, lstart = item
                k0, kw = KB[kb]
                for i in range(2):
                    h = 2 * hp + i
                    nc.tensor.matmul(
                        avs[i][0:65, lstart:cw],
                        v_r[0:kw, kb, h, :],
                        att[0:kw, i * sw + off:i * sw + off + iw],
                        start=(kb == 0), stop=(kb == nkb - 1))

            def emit_scores_plain(p):
                wl = widths(p[0])
                sw = sum(w for _, _, w, _ in wl)
                att = att_pool.tile([128, 2 * sw], BF16, tag="att")
                for grp in groups_of(wl):
                    emit_score_group(p, att, sw, grp)
                state[("att", p)] = (att, sw)

            def emit_av_scores(p_av, p_sco, use_psP=False):
                """av MMs of p_av interleaved (PE-queue) with score MMs of
                p_sco so exp-paced score stalls are absorbed by av work."""
                av_items = widths(p_av[0]) if p_av else []
                nkb = len(av_items)
                if p_av:
                    att, sw_a = state.pop(("att", p_av))
                    # late pairs borrow the proj pool (its chains are done
                    # by then): consecutive pairs then alternate PSUM
                    # pools, hiding the evacuation-cast latency
                    pool, tg = (psP, "ps_proj") if use_psP else \
                               (psAv, "ps_av")
                    avs = [pool.tile([128, 512], F32, tag=tg,
                                     name=f"av{i}") for i in range(2)]
                    state[("avs", p_av)] = avs
                sco_groups = groups_of(widths(p_sco[0])) if p_sco else []
                if p_sco:
                    sw_s = sum(w for _, _, w, _ in widths(p_sco[0]))
                    att_s = att_pool.tile([128, 2 * sw_s], BF16, tag="att")
                    state[("att", p_sco)] = (att_s, sw_s)
                # av items lag the score groups by 2 so the av-chain start
                # (which waits on the psAv banks' evacuation) sits behind
                # already-runnable score work in the PE queue
                L = 2
                for t in range(max(len(sco_groups), L + len(av_items))):
                    if t < len(sco_groups):
                        emit_score_group(p_sco, att_s, sw_s, sco_groups[t])
                    if p_av and 0 <= t - L < len(av_items):
                        emit_av_kb(p_av, att, sw_a, avs, av_items[t - L],
                                   nkb)

            def emit_out(p):
                scn, hp = p
                q0, cw = QC[scn]
                avs = state.pop(("avs", p))
                for i in range(2):
                    h = 2 * hp + i
                    o_sb = out_pool.tile([65, CH], BF16, tag="osb",
                                         name="o_sb")
                    # both casts on DVE: ScalarE is the locally saturated
                    # engine in the exp-heavy middle slots
                    nc.vector.tensor_copy(o_sb[:, 0:cw], avs[i][0:65, 0:cw])
                    nc.sync.dma_start(
                        outT[h * 65:(h + 1) * 65, q0:q0 + cw],
                        o_sb[:, 0:cw])

            # ---------------- schedule ----------------
            # One upfront 8-chain group (borrowing every PSUM bank): all
            # k+q ch0 chains walk the eb chunks together, so the PE
            # streams densely at DMA pace and HAM warms early.
            emit_group([("k", 0, 0), ("q", 0, 0), ("k", 1, 0), ("q", 1, 0),
                        ("k", 2, 0), ("q", 2, 0), ("k", 3, 0), ("q", 3, 0)])

            # remaining chains spread over attention slots (need-by safe:
            # S(c,hp) needs (k+q)(ob=hp, ch<=c) two slots early; av(c,*)
            # needs v blocks < visible range by its own slot)
            def G(*chains):
                return lambda: emit_group(list(chains))
            sched = {}
            if NCH == 3 and NB == 9:
                # chunk1/chunk2 pairs alternate so the heavy chunk-1 exps
                # (~7.6us) interleave with the tiny chunk-2 ones on
                # ScalarE. Need-by: scores(c,hp) two slots ahead of av
                # need (k+q)(ob=hp, ch<=c); av(1,0)@slot4 reads v<=7,
                # av(2,0)@slot5 reads v8 -- all v chains land a slot early.
                pairs = [(0, 0), (0, 1), (0, 2), (0, 3),
                         (1, 0), (2, 0), (1, 1), (2, 1),
                         (1, 2), (2, 2), (1, 3), (2, 3)]
                sched = {
                    (0, 0): [G(("k", 0, 1), ("q", 0, 1)), G(("v", 4))],
                    (0, 1): [G(("k", 1, 1), ("q", 1, 1)), G(("v", 5))],
                    (0, 2): [G(("k", 0, 2), ("q", 0, 2)), G(("v", 6))],
                    (0, 3): [G(("k", 1, 2), ("q", 1, 2)), G(("v", 7))],
                    (1, 0): [G(("k", 2, 1), ("q", 2, 1)), G(("v", 8))],
                    (2, 0): [G(("k", 2, 2), ("q", 2, 2))],
                    (1, 1): [G(("k", 3, 1), ("q", 3, 1))],
                    (2, 1): [G(("k", 3, 2), ("q", 3, 2))],
                }
            else:
                for ch in range(1, NCH):
                    for ob in range(4):
                        emit_group([("k", ob, ch), ("q", ob, ch)])
                for sb in range(3, NB):
                    emit_group([("v", sb)])
                pairs = [(scn, hp) for scn in range(NCH)
                         for hp in range(4)]
            n = len(pairs)
            emit_scores_plain(pairs[0])
            emit_scores_plain(pairs[1])
            # v blocks 0-3 after the first score blocks: PE starts attention
            # sooner; av(0,0) reads kb 0..3 so all four must precede slot 0
            emit_group([("v", 0), ("v", 1)])
            emit_group([("v", 2), ("v", 3)])
            for j, p in enumerate(pairs):
                for fn in sched.get(p, ()):
                    fn()
                emit_av_scores(p, pairs[j + 2] if j + 2 < n else None,
                               use_psP=(j >= 9))
                emit_out(p)

    nc.compile()
    return nc


def get_nc(seqc=CAP0):
    if seqc not in _cache:
        _cache[seqc] = _build_nc(seqc)
    return _cache[seqc]


def _prep(input_x, pad_mask, Wq, bq, Wk, bk, Wv, bv):
    import ml_dtypes
    bf16 = ml_dtypes.bfloat16
    input_x = np.asarray(input_x, dtype=np.float32)
    pad = np.asarray(pad_mask)
    Ws = [np.asarray(w, dtype=np.float32) for w in (Wq, Wk, Wv)]
    bs = [np.ascontiguousarray(np.asarray(v, dtype=np.float32))
          for v in (bq, bk, bv)]

    idxs = [np.flatnonzero(pad[b]) for b in range(B)]
    sbs = [len(ix) for ix in idxs]
    cap = max(CAP0, -(-max(sbs) // 64) * 64)

    xTs = []
    for b in range(B):
        xc = np.zeros((cap, E), np.float32)
        xc[:sbs[b]] = input_x[b][idxs[b]]
        xTs.append(np.ascontiguousarray(xc.T).astype(bf16))

    wslices = {}
    for g in range(2):
        sl = slice(g * OC, (g + 1) * OC)
        bqk = np.ascontiguousarray(np.concatenate(
            [bs[0][sl].reshape(4, 128), bs[1][sl].reshape(4, 128)], axis=0))
        wslices[g] = tuple(
            np.ascontiguousarray(W[sl].T).astype(bf16) for W in Ws
        ) + (bqk, np.ascontiguousarray(bs[2][sl]))

    in_maps = []
    for c in range(NCORES):
        b, g = c // 2, c % 2
        wq_t, wk_t, wv_t, bqk_s, bv_s = wslices[g]
        in_maps.append({
            "xT": xTs[b], "wqT": wq_t, "wkT": wk_t, "wvT": wv_t,
            "bqk": bqk_s, "bv": bv_s,
        })
    return in_maps, idxs, sbs, cap


def _assemble(results, idxs, sbs, S):
    out = np.zeros((B, S, E), dtype=np.float32)
    for c in range(NCORES):
        b, g = c // 2, c % 2
        arr = np.asarray(results[c]["outT"], dtype=np.float32)  # [520, cap]
        nb = sbs[b]
        for h in range(HPC):
            blk = arr[h * 65:(h + 1) * 65, :nb]
            o = blk[:64] / blk[64:65]
            out[b, idxs[b], g * OC + h * 64:g * OC + (h + 1) * 64] = o.T
    return out


LAST_RESULT = None


def kernel(input_x, pad_mask, Wq, bq, Wk, bk, Wv, bv):
    from concourse.bass_utils import run_bass_kernel_spmd
    global LAST_RESULT
    S = np.asarray(input_x).shape[1]
    in_maps, idxs, sbs, cap = _prep(input_x, pad_mask, Wq, bq, Wk, bk, Wv, bv)
    nc = get_nc(cap)
    res = run_bass_kernel_spmd(nc, in_maps, core_ids=list(range(NCORES)))
    LAST_RESULT = res
    if res.exec_time_ns is not None:
        print(f"HW exec time: {res.exec_time_ns} ns")
    return _assemble(res.results, idxs, sbs, S)


# revision 32
# speedup vs baseline: 1.0190x; 1.0190x over previous
"""Multi-head causal+padded attention on 8 Trainium2 NeuronCores.

Core c handles batch b = c//2 and head-group g = c%2 (8 of 16 heads).

Pad compaction: the reference masks out padded keys/queries entirely
(padded query rows output 0). Attention over the pad-compacted sequence is
exactly equivalent, so the host gathers the ~1024 unpadded rows per batch,
zero-pads to a fixed 1088 capacity, and the device runs a causal MHA on
[1088]. Outputs are scattered back with zeros in padded rows.

Device (per core, all-bf16 datapath, fp32 PSUM):
  qT/kT = W^T-slices @ xT in [out, seq] layout; v in natural [seq, out]
  layout augmented with a ones column (softmax denominator rides along the
  att@v accumulation chain). Scores transposed per 128-k-block, head pairs
  packed into PE row-groups 0-1/2-3 (concurrent matmuls); consecutive
  k-blocks share PSUM tiles so one batched exp serves several blocks;
  tri-masked on GpSimd, then att^T-chained into [65, CH] PSUM; DVE
  evacuates unnormalized out+denominator as bf16; the host divides.

  Startup: input DMA fans out over the three DMA-capable queues (x split
  over sync+gpsimd, wk/wq interleaved by eb on scalar so the paired k+q
  ob-chains stream at DMA pace); the per-partition q/k biases arrive as
  one contiguous [8,128] row-DMA and are PE-transposed on the otherwise
  idle tensor engine. Upfront projection groups pair k+q chains per
  ob-block so the first score pair (which only needs the ob0 slices)
  starts as early as possible. Emission is software-pipelined with stage
  offsets: slot j emits [proj chains | exp+mask(j+1) | scores(j+2)
  interleaved with av(j) | out(j)].
"""
import sys

sys.path.insert(0, "/opt/trn_rl_repo")

import numpy as np

E = 1024
D = 64
HPC = 8         # heads per core
OC = HPC * D    # 512 output dims per core
EB = E // 128   # 8 contraction blocks
B = 4
NCORES = 8
CH = 512        # max q-chunk width (PSUM-bank exact)
CAP0 = 1088     # default compacted seq capacity (multiple of 64)

_cache = {}


def _qchunks(seqc):
    out, q0 = [], 0
    while q0 < seqc:
        w = min(CH, seqc - q0)
        out.append((q0, w))
        q0 += w
    return out


def _kblocks(seqc):
    out, k0 = [], 0
    while k0 < seqc:
        w = min(128, seqc - k0)
        out.append((k0, w))
        k0 += w
    return out


def _build_nc(seqc):
    from concourse import bacc
    import concourse.tile as tile
    import concourse.mybir as mybir

    assert seqc % 64 == 0
    QC = _qchunks(seqc)          # [(q0, cw)]
    KB = _kblocks(seqc)          # [(k0, kw)]
    NCH = len(QC)
    NB = len(KB)
    F32 = mybir.dt.float32
    BF16 = mybir.dt.bfloat16
    AF = mybir.ActivationFunctionType

    nc = bacc.Bacc("TRN2", target_bir_lowering=False, debug=False,
                   num_devices=NCORES)
    xT = nc.dram_tensor("xT", [E, seqc], BF16, kind="ExternalInput").ap()
    wqT = nc.dram_tensor("wqT", [E, OC], BF16, kind="ExternalInput").ap()
    wkT = nc.dram_tensor("wkT", [E, OC], BF16, kind="ExternalInput").ap()
    wvT = nc.dram_tensor("wvT", [E, OC], BF16, kind="ExternalInput").ap()
    # bq/bk stacked as [8, 128]: rows 0-3 = bq.reshape(4,128), 4-7 = bk
    bqk = nc.dram_tensor("bqk", [8, 128], F32, kind="ExternalInput").ap()
    bv = nc.dram_tensor("bv", [OC], F32, kind="ExternalInput").ap()
    # unnormalized out (64 rows) + denominator (row 64) per head
    outT = nc.dram_tensor("outT", [HPC * 65, seqc], BF16,
                          kind="ExternalOutput").ap()

    with tile.TileContext(nc) as tc:
        with tc.tile_pool(name="const", bufs=1) as cpool, \
             tc.tile_pool(name="big", bufs=1) as bigpool, \
             tc.tile_pool(name="psP", bufs=2, space="PSUM") as psP, \
             tc.tile_pool(name="psS", bufs=2, space="PSUM") as psS, \
             tc.tile_pool(name="psAv", bufs=2, space="PSUM") as psAv, \
             tc.tile_pool(name="att", bufs=5) as att_pool, \
             tc.tile_pool(name="outp", bufs=6) as out_pool:

            # ---------------- persistent SBUF ----------------
            x_sb = bigpool.tile([128, EB * seqc], BF16, tag="x_sb")
            wq_sb = bigpool.tile([128, EB * OC], BF16, tag="wq_sb")
            wk_sb = bigpool.tile([128, EB * OC], BF16, tag="wk_sb")
            wv_sb = bigpool.tile([128, EB * OC], BF16, tag="wv_sb")
            qT_sb = bigpool.tile([128, 4 * seqc], BF16, tag="qT")
            kT_sb = bigpool.tile([128, 4 * seqc], BF16, tag="kT")
            v_aug = bigpool.tile([128, NB * HPC * 65], BF16, tag="v_aug")
            v_r = v_aug[:].rearrange("p (b h c) -> p b h c", b=NB, h=HPC)

            # tri[k, q] = 1 where k <= q else 0 (diagonal 128x128 block);
            # memset early so the warm exp below has a ready operand
            tri = cpool.tile([128, 128], BF16, tag="tri")
            nc.gpsimd.memset(tri[:], 1.0)
            # ~4us of dummy matmuls while the input DMA streams: HAM
            # un-throttles the PE clock (1.2 -> 2.4 GHz) after ~3.4us of
            # sustained activity, so the real chains start warm
            dumm = cpool.tile([128, 512], BF16, tag="dumm")
            nc.gpsimd.memset(dumm[:], 0.0)
            warm_ps = psP.tile([128, 512], F32, tag="ps_proj", name="warm")
            for _ in range(10):
                nc.tensor.matmul(warm_ps[:], dumm[:, 0:128], dumm[:],
                                 start=True, stop=True)
            # hoist the exp ACT_TABLE_LOAD (~2.7us) into the DMA window;
            # scale=0.0 makes the operand value irrelevant
            warm = cpool.tile([1, 4], F32, tag="warm")
            nc.scalar.activation(warm[:], tri[0:1, 0:4], AF.Exp, scale=0.0)
            # 8x8 identity for the PE-transpose of the stacked q/k biases
            id8 = cpool.tile([8, 8], F32, tag="id8")
            nc.gpsimd.memset(id8[:], 0.0)
            nc.gpsimd.affine_select(
                out=id8[:], in_=id8[:],
                compare_op=mybir.AluOpType.not_equal, fill=1.0,
                base=0, pattern=[[-1, 8]], channel_multiplier=1)

            # The upfront 8-chain group consumes (x, wk, wq)[eb] per eb
            # step, so the gating 4.36MB is spread evenly over the three
            # DMA queues roughly in eb order; wv (not needed until the
            # first v chains, several slots in) strictly follows.
            def dma_w(eng, w_sb, wT, eb):
                eng.dma_start(w_sb[:, eb * OC:(eb + 1) * OC],
                              wT[eb * 128:(eb + 1) * 128, :])

            def dma_x(eng, eb):
                eng.dma_start(x_sb[:, eb * seqc:(eb + 1) * seqc],
                              xT[eb * 128:(eb + 1) * 128, :])

            def dma_xh(eng, eb, half):
                c0, c1 = (0, 512) if half == 0 else (512, seqc)
                eng.dma_start(x_sb[:, eb * seqc + c0:eb * seqc + c1],
                              xT[eb * 128:(eb + 1) * 128, c0:c1])

            # gating set (x cols 0:512 + wk + wq = what the upfront ch0
            # chains consume) round-robins the three queues in eb order
            # so per-eb triples land together; the rest streams behind
            ENG = (nc.sync, nc.scalar, nc.gpsimd)
            bqk_sb = cpool.tile([8, 128], F32, tag="bqk")
            nc.sync.dma_start(bqk_sb[:], bqk)
            for eb in range(EB):
                dma_xh(ENG[eb % 3], eb, 0)
                dma_w(ENG[(eb + 1) % 3], wk_sb, wkT, eb)
                dma_w(ENG[(eb + 2) % 3], wq_sb, wqT, eb)
            bv_row = cpool.tile([1, OC], F32, tag="bv_row")
            nc.sync.dma_start(bv_row[:], bv.rearrange("(a c) -> a c", a=1))
            for eb in range(EB):
                dma_xh(ENG[(eb + 1) % 3], eb, 1)
                dma_w(ENG[(eb + 2) % 3], wv_sb, wvT, eb)

            # biases: PE-transpose [8,128] -> [128,8] on the idle PE queue
            bqk_T = cpool.tile([128, 8], F32, tag="bqkT")
            ps_b = psP.tile([128, 512], F32, tag="ps_proj", name="ps_b")
            nc.tensor.transpose(ps_b[:, 0:8], bqk_sb[:], id8[:])
            nc.vector.tensor_copy(bqk_T[:], ps_b[:, 0:8])
            bq_sb = bqk_T[:, 0:4]
            bk_sb = bqk_T[:, 4:8]

            nc.gpsimd.affine_select(
                out=tri[:], in_=tri[:], compare_op=mybir.AluOpType.is_ge,
                fill=0.0, base=0, pattern=[[1, 128]], channel_multiplier=-1)
            bv_tile = cpool.tile([128, OC], F32, tag="bv_tile")
            nc.gpsimd.partition_broadcast(bv_tile[:], bv_row[:])
            nc.gpsimd.memset(v_r[:, :, :, 64], 1.0)

            # ---------------- projection chain groups ----------------
            # chain spec: ("k"|"q", ob, ch) weight-stationary, or ("v", sb)
            def emit_group(chains):
                # PSUM banks per chain: 2x psP, then loans of psS pair
                # tiles (2 banks each) and psAv tiles for the startup
                # 8-chain group; attention hasn't started yet then
                pss = []
                loans = {}
                for idx, c in enumerate(chains):
                    if idx < 2:
                        ps = psP.tile([128, 512], F32, tag="ps_proj",
                                      name=f"pp{idx}")
                    elif idx < 6:
                        li = (idx - 2) // 2
                        if li not in loans:
                            loans[li] = psS.tile([128, 1024], F32,
                                                 tag="ps_s", name="loan")
                        ps = loans[li][:, (idx % 2) * 512:
                                       (idx % 2 + 1) * 512]
                    else:
                        ps = psAv.tile([128, 512], F32, tag="ps_av",
                                       name=f"av{idx - 6}")
                    pss.append(ps)
                for eb in range(EB):
                    for c, ps in zip(chains, pss):
                        if c[0] == "v":
                            sb = c[1]
                            k0, kw = KB[sb]
                            nc.tensor.matmul(
                                ps[0:kw, :],
                                x_sb[:, eb * seqc + k0:eb * seqc + k0 + kw],
                                wv_sb[:, eb * OC:(eb + 1) * OC],
                                start=(eb == 0), stop=(eb == EB - 1))
                        else:
                            _, ob, ch = c
                            q0, cw = QC[ch]
                            w_sb = wk_sb if c[0] == "k" else wq_sb
                            nc.tensor.matmul(
                                ps[:, 0:cw],
                                w_sb[:, eb * OC + ob * 128:
                                     eb * OC + (ob + 1) * 128],
                                x_sb[:, eb * seqc + q0:eb * seqc + q0 + cw],
                                start=(eb == 0), stop=(eb == EB - 1))
                for c, ps in zip(chains, pss):
                    if c[0] == "v":
                        sb = c[1]
                        k0, kw = KB[sb]
                        nc.vector.tensor_add(
                            v_r[0:kw, sb, :, 0:64],
                            ps[0:kw, :].rearrange("p (h c) -> p h c", h=HPC),
                            bv_tile[0:kw, :].rearrange("p (h c) -> p h c",
                                                       h=HPC))
                    else:
                        _, ob, ch = c
                        q0, cw = QC[ch]
                        dst = kT_sb if c[0] == "k" else qT_sb
                        bias_sb = bk_sb if c[0] == "k" else bq_sb
                        nc.vector.tensor_scalar_add(
                            dst[:, ob * seqc + q0:ob * seqc + q0 + cw],
                            ps[:, 0:cw], bias_sb[:, ob:ob + 1])

            # ---------------- attention emitters ----------------
            def widths(scn):
                """[(kb, off, iw, lstart)] for chunk scn, packed offsets."""
                q0, cw = QC[scn]
                out, off = [], 0
                for kb, (k0, kw) in enumerate(KB):
                    if k0 >= q0 + cw:
                        break
                    lstart = max(0, k0 - q0)
                    iw = cw - lstart
                    out.append((kb, off, iw, lstart))
                    off += iw
                return out

            def groups_of(items):
                """Pack consecutive items into <=512-col groups sharing a
                PSUM tile + one exp; break when the k-width changes."""
                gs, cur, w = [], [], 0
                for it in items:
                    kw = KB[it[0]][1]
                    if cur and (w + it[2] > 512 or KB[cur[0][0]][1] != kw):
                        gs.append(cur)
                        cur, w = [], 0
                    cur.append(it)
                    w += it[2]
                if cur:
                    gs.append(cur)
                return gs

            state = {}

            def emit_score_group(p, att, sw, grp):
                """Paired score MMs per item (head pair -> the two banks of
                one PSUM tile), then ONE fused strided exp PSUM->SBUF for
                the whole group, then gpsimd tri-masks on diagonal items."""
                scn, hp = p
                q0, cw = QC[scn]
                kw = KB[grp[0][0]][1]
                gw = sum(it[2] for it in grp)
                off0 = grp[0][1]
                ssb = psS.tile([128, 1024], F32, tag="ps_s")
                goff = 0
                for kb, off, iw, lstart in grp:
                    k0, _ = KB[kb]
                    for i in range(2):
                        h = 2 * hp + i
                        ob, po = h // 2, (h % 2) * 64
                        # one accumulation epoch per bank: start only on the
                        # group's first item (first_mm clears the whole
                        # bank's has_written bits), stop on its last
                        nc.tensor.matmul(
                            ssb[0:kw, i * 512 + goff:i * 512 + goff + iw],
                            kT_sb[po:po + 64,
                                  ob * seqc + k0:ob * seqc + k0 + kw],
                            qT_sb[po:po + 64,
                                  ob * seqc + q0 + lstart:
                                  ob * seqc + q0 + cw],
                            start=(goff == 0), stop=(goff + iw == gw))
                    goff += iw
                src = ssb[:].rearrange("p (i c) -> p i c", i=2)[0:kw, :, 0:gw]
                dst = att[:].rearrange("p (i c) -> p i c",
                                       i=2)[0:kw, :, off0:off0 + gw]
                nc.scalar.activation(dst, src, AF.Exp, scale=0.125)
                for kb, off, iw, lstart in grp:
                    k0, bkw = KB[kb]
                    if k0 >= q0:  # diagonal block: causal tri mask
                        mw = min(bkw, iw)
                        for i in range(2):
                            # split across GpSimd and DVE: two engines
                            # drain the exp->mask->av chain in parallel
                            eng = nc.gpsimd if i == 0 else nc.vector
                            eng.tensor_mul(
                                att[0:bkw, i * sw + off:i * sw + off + mw],
                                att[0:bkw, i * sw + off:i * sw + off + mw],
                                tri[0:bkw, 0:mw])

            def emit_av_kb(p, att, sw, avs, item, nkb):
                scn, hp = p
                q0, cw = QC[scn]
                kb, off, iw, lstart = item
                k0, kw = KB[kb]
                for i in range(2):
                    h = 2 * hp + i
                    nc.tensor.matmul(
                        avs[i][0:65, lstart:cw],
                        v_r[0:kw, kb, h, :],
                        att[0:kw, i * sw + off:i * sw + off + iw],
                        start=(kb == 0), stop=(kb == nkb - 1))

            def emit_scores_plain(p):
                wl = widths(p[0])
                sw = sum(w for _, _, w, _ in wl)
                att = att_pool.tile([128, 2 * sw], BF16, tag="att")
                for grp in groups_of(wl):
                    emit_score_group(p, att, sw, grp)
                state[("att", p)] = (att, sw)

            def emit_av_scores(p_av, p_sco, use_psP=False):
                """av MMs of p_av interleaved (PE-queue) with score MMs of
                p_sco so exp-paced score stalls are absorbed by av work."""
                av_items = widths(p_av[0]) if p_av else []
                nkb = len(av_items)
                if p_av:
                    att, sw_a = state.pop(("att", p_av))
                    # late pairs borrow the proj pool (its chains are done
                    # by then): consecutive pairs then alternate PSUM
                    # pools, hiding the evacuation-cast latency
                    pool, tg = (psP, "ps_proj") if use_psP else \
                               (psAv, "ps_av")
                    avs = [pool.tile([128, 512], F32, tag=tg,
                                     name=f"av{i}") for i in range(2)]
                    state[("avs", p_av)] = avs
                sco_groups = groups_of(widths(p_sco[0])) if p_sco else []
                if p_sco:
                    sw_s = sum(w for _, _, w, _ in widths(p_sco[0]))
                    att_s = att_pool.tile([128, 2 * sw_s], BF16, tag="att")
                    state[("att", p_sco)] = (att_s, sw_s)
                # av items lag the score groups by 2 so the av-chain start
                # (which waits on the psAv banks' evacuation) sits behind
                # already-runnable score work in the PE queue
                L = 2
                for t in range(max(len(sco_groups), L + len(av_items))):
                    if t < len(sco_groups):
                        emit_score_group(p_sco, att_s, sw_s, sco_groups[t])
                    if p_av and 0 <= t - L < len(av_items):
                        emit_av_kb(p_av, att, sw_a, avs, av_items[t - L],
                                   nkb)

            def emit_out(p):
                scn, hp = p
                q0, cw = QC[scn]
                avs = state.pop(("avs", p))
                for i in range(2):
                    h = 2 * hp + i
                    o_sb = out_pool.tile([65, CH], BF16, tag="osb",
                                         name="o_sb")
                    # both casts on DVE: ScalarE is the locally saturated
                    # engine in the exp-heavy middle slots
                    nc.vector.tensor_copy(o_sb[:, 0:cw], avs[i][0:65, 0:cw])
                    nc.sync.dma_start(
                        outT[h * 65:(h + 1) * 65, q0:q0 + cw],
                        o_sb[:, 0:cw])

            # ---------------- schedule ----------------
            # One upfront 8-chain group (borrowing every PSUM bank): all
            # k+q ch0 chains walk the eb chunks together, so the PE
            # streams densely at DMA pace and HAM warms early.
            emit_group([("k", 0, 0), ("q", 0, 0), ("k", 1, 0), ("q", 1, 0),
                        ("k", 2, 0), ("q", 2, 0), ("k", 3, 0), ("q", 3, 0)])

            # remaining chains spread over attention slots (need-by safe:
            # S(c,hp) needs (k+q)(ob=hp, ch<=c) two slots early; av(c,*)
            # needs v blocks < visible range by its own slot)
            def G(*chains):
                return lambda: emit_group(list(chains))
            sched = {}
            if NCH == 3 and NB == 9:
                # chunk1/chunk2 pairs alternate so the heavy chunk-1 exps
                # (~7.6us) interleave with the tiny chunk-2 ones on
                # ScalarE. Need-by: scores(c,hp) two slots ahead of av
                # need (k+q)(ob=hp, ch<=c); av(1,0)@slot4 reads v<=7,
                # av(2,0)@slot5 reads v8 -- all v chains land a slot early.
                pairs = [(0, 0), (0, 1), (0, 2), (0, 3),
                         (1, 0), (2, 0), (1, 1), (2, 1),
                         (1, 2), (2, 2), (1, 3), (2, 3)]
                sched = {
                    (0, 0): [G(("k", 0, 1), ("q", 0, 1)), G(("v", 4))],
                    (0, 1): [G(("k", 1, 1), ("q", 1, 1)), G(("v", 5))],
                    (0, 2): [G(("k", 0, 2), ("q", 0, 2)), G(("v", 6))],
                    (0, 3): [G(("k", 1, 2), ("q", 1, 2)), G(("v", 7))],
                    (1, 0): [G(("k", 2, 1), ("q", 2, 1)), G(("v", 8))],
                    (2, 0): [G(("k", 2, 2), ("q", 2, 2))],
                    (1, 1): [G(("k", 3, 1), ("q", 3, 1))],
                    (2, 1): [G(("k", 3, 2), ("q", 3, 2))],
                }
            else:
                for ch in range(1, NCH):
                    for ob in range(4):
                        emit_group([("k", ob, ch), ("q", ob, ch)])
                for sb in range(3, NB):
                    emit_group([("v", sb)])
                pairs = [(scn, hp) for scn in range(NCH)
                         for hp in range(4)]
            n = len(pairs)
            emit_scores_plain(pairs[0])
            emit_scores_plain(pairs[1])
            # v blocks 0-3 after the first score blocks: PE starts attention
            # sooner; av(0,0) reads kb 0..3 so all four must precede slot 0
            emit_group([("v", 0), ("v", 1)])
            emit_group([("v", 2), ("v", 3)])
            for j, p in enumerate(pairs):
                for fn in sched.get(p, ()):
                    fn()
                emit_av_scores(p, pairs[j + 2] if j + 2 < n else None)
                emit_out(p)

    nc.compile()
    return nc


def get_nc(seqc=CAP0):
    if seqc not in _cache:
        _cache[seqc] = _build_nc(seqc)
    return _cache[seqc]


def _prep(input_x, pad_mask, Wq, bq, Wk, bk, Wv, bv):
    import ml_dtypes
    bf16 = ml_dtypes.bfloat16
    input_x = np.asarray(input_x, dtype=np.float32)
    pad = np.asarray(pad_mask)
    Ws = [np.asarray(w, dtype=np.float32) for w in (Wq, Wk, Wv)]
    bs = [np.ascontiguousarray(np.asarray(v, dtype=np.float32))
          for v in (bq, bk, bv)]

    idxs = [np.flatnonzero(pad[b]) for b in range(B)]
    sbs = [len(ix) for ix in idxs]
    cap = max(CAP0, -(-max(sbs) // 64) * 64)

    xTs = []
    for b in range(B):
        xc = np.zeros((cap, E), np.float32)
        xc[:sbs[b]] = input_x[b][idxs[b]]
        xTs.append(np.ascontiguousarray(xc.T).astype(bf16))

    wslices = {}
    for g in range(2):
        sl = slice(g * OC, (g + 1) * OC)
        bqk = np.ascontiguousarray(np.concatenate(
            [bs[0][sl].reshape(4, 128), bs[1][sl].reshape(4, 128)], axis=0))
        wslices[g] = tuple(
            np.ascontiguousarray(W[sl].T).astype(bf16) for W in Ws
        ) + (bqk, np.ascontiguousarray(bs[2][sl]))

    in_maps = []
    for c in range(NCORES):
        b, g = c // 2, c % 2
        wq_t, wk_t, wv_t, bqk_s, bv_s = wslices[g]
        in_maps.append({
            "xT": xTs[b], "wqT": wq_t, "wkT": wk_t, "wvT": wv_t,
            "bqk": bqk_s, "bv": bv_s,
        })
    return in_maps, idxs, sbs, cap


def _assemble(results, idxs, sbs, S):
    out = np.zeros((B, S, E), dtype=np.float32)
    for c in range(NCORES):
        b, g = c // 2, c % 2
        arr = np.asarray(results[c]["outT"], dtype=np.float32)  # [520, cap]
        nb = sbs[b]
        for h in range(HPC):
            blk = arr[h * 65:(h + 1) * 65, :nb]
            o = blk[:64] / blk[64:65]
            out[b, idxs[b], g * OC + h * 64:g * OC + (h + 1) * 64] = o.T
    return out


LAST_RESULT = None


def kernel(input_x, pad_mask, Wq, bq, Wk, bk, Wv, bv):
    from concourse.bass_utils import run_bass_kernel_spmd
    global LAST_RESULT
    S = np.asarray(input_x).shape[1]
    in_maps, idxs, sbs, cap = _prep(input_x, pad_mask, Wq, bq, Wk, bk, Wv, bv)
    nc = get_nc(cap)
    res = run_bass_kernel_spmd(nc, in_maps, core_ids=list(range(NCORES)))
    LAST_RESULT = res
    if res.exec_time_ns is not None:
        print(f"HW exec time: {res.exec_time_ns} ns")
    return _assemble(res.results, idxs, sbs, S)
